# revision 9
# baseline (speedup 1.0000x reference)
"""HRR self-attention kernel for 8 Trainium2 NeuronCores.

Math (per reference):
  q = x @ W_q.T ; kv = x @ W_kv.T                (B,S,H,hd) heads, hd=128
  qf = fft(q) / ||fft(q)|| ; kvf = fft(kv)/||.|| (norm over head dim)
  kvc = cumsum(kvf, seq) ; out = ifft(qf*kvc).real @ W_o.T

Device mapping: FFT/IFFT as 128x128 DFT matmuls (Layout B: head-dim/freq on
partitions, tokens on free dim). Norms via Parseval in the time domain
(ones-matmul partition reduction, broadcast for free). Cumsum via the DVE
tensor_tensor_scan along the free dim. Sharding: 8 cores = (batch, seq-half),
2048 tokens/core; cross-core cumsum prefix fixed up with a 16KB pair
AllGather folded into the complex multiply as a scalar_tensor_tensor bias.
"""
import sys

sys.path.insert(0, "/opt/trn_rl_repo")

import numpy as np
import ml_dtypes

import concourse.bass as bass
import concourse.bacc as bacc
import concourse.tile as tile
import concourse.mybir as mybir
from concourse.bass_utils import run_bass_kernel_spmd

BF = ml_dtypes.bfloat16
F32 = mybir.dt.float32
BF16 = mybir.dt.bfloat16
AF = mybir.ActivationFunctionType
ALU = mybir.AluOpType

B, S, D = 4, 4096, 2048
H, HD = 16, 128
TOK = 2048          # tokens per core
KT = D // 128       # 16 k-tiles
N_CORES = 8
CH = 1024           # psum chunk (2 banks)

_CACHE = {}


def _build():
    import os
    PHASES = os.environ.get("KPHASES", "KQO")
    nc = bacc.Bacc("TRN2", target_bir_lowering=False, debug=False,
                   num_devices=N_CORES)
    xT = nc.dram_tensor("xT", [D, TOK], BF16, kind="ExternalInput").ap()
    WqT = nc.dram_tensor("WqT", [D, D], BF16, kind="ExternalInput").ap()
    WkvT = nc.dram_tensor("WkvT", [D, D], BF16, kind="ExternalInput").ap()
    WoT = nc.dram_tensor("WoT", [D, D], BF16, kind="ExternalInput").ap()
    Cm_d = nc.dram_tensor("Cm", [HD, HD], BF16, kind="ExternalInput").ap()
    Sn_d = nc.dram_tensor("Sn", [HD, HD], BF16, kind="ExternalInput").ap()
    Am_d = nc.dram_tensor("Am", [HD, HD], BF16, kind="ExternalInput").ap()
    Bn_d = nc.dram_tensor("Bn", [HD, HD], BF16, kind="ExternalInput").ap()
    ones_d = nc.dram_tensor("ones", [HD, HD], BF16, kind="ExternalInput").ap()
    mask_d = nc.dram_tensor("mask", [HD, 2 * H], F32, kind="ExternalInput").ap()
    out = nc.dram_tensor("out", [TOK, D], F32, kind="ExternalOutput").ap()

    with tile.TileContext(nc) as tc:
        with (
            tc.tile_pool(name="const", bufs=1) as cp,
            tc.tile_pool(name="dram", bufs=1, space="DRAM") as dp,
        ):
            cm = cp.tile([HD, HD], BF16)
            sn = cp.tile([HD, HD], BF16)
            am = cp.tile([HD, HD], BF16)
            bn = cp.tile([HD, HD], BF16)
            ones = cp.tile([HD, HD], BF16)
            mask = cp.tile([HD, 2 * H], F32)
            loc = cp.tile([HD, 2 * H], F32)
            part = cp.tile([HD, 2 * H], F32)
            offs = cp.tile([HD, 2 * H], F32)
            zeros = cp.tile([128, CH], BF16)
            for t, d_ in ((cm, Cm_d), (sn, Sn_d), (am, Am_d), (bn, Bn_d),
                          (ones, ones_d), (mask, mask_d)):
                nc.sync.dma_start(t[:], d_)
            nc.vector.memset(zeros[:], 0.0)

            kvc_d = dp.tile([H, 128, TOK], BF16)   # re
            kvci_d = dp.tile([H, 128, TOK], BF16)  # im
            vh_d = dp.tile([H, 128, TOK], BF16)
            cc_in = dp.tile([HD, 2 * H], F32)
            cc_out = dp.tile([2 * HD, 2 * H], F32)

            with (
                tc.tile_pool(name="xw", bufs=1) as xp,
                tc.tile_pool(name="work", bufs=1) as wp,
                tc.tile_pool(name="ps", bufs=1, space="PSUM") as pp,
            ):
                xsb = xp.tile([128, KT * TOK], BF16)
                for k in range(KT):
                    nc.sync.dma_start(xsb[:, k * TOK:(k + 1) * TOK],
                                      xT[k * 128:(k + 1) * 128, :])

                def proj_norm(h, w_ap, tag):
                    """q/kv projection for head h + time-domain normalize.
                    Returns normalized [128, TOK] bf16 SBUF tile."""
                    wsl = xp.tile([128, KT * 128], BF16, tag="wsl", bufs=3)
                    nc.sync.dma_start(
                        wsl[:].rearrange("p (k n) -> p k n", k=KT),
                        w_ap[:, h * 128:(h + 1) * 128].rearrange(
                            "(k p) n -> p k n", p=128))
                    nrm = wp.tile([128, TOK], BF16, tag=f"nrm{tag}", bufs=2)
                    for c in range(TOK // CH):
                        ps = pp.tile([128, CH], F32, tag="pk", bufs=2)
                        for k in range(KT):
                            lt = wsl[:, k * 128:(k + 1) * 128]
                            for nb in range(CH // 512):
                                col = c * CH + nb * 512
                                nc.tensor.matmul(
                                    ps[:, nb * 512:(nb + 1) * 512], lt,
                                    xsb[:, k * TOK + col: k * TOK + col + 512],
                                    start=(k == 0), stop=(k == KT - 1))
                        sq = wp.tile([128, CH], BF16, tag="sq", bufs=2)
                        nc.scalar.activation(sq[:], ps[:], AF.Square)
                        nn = pp.tile([128, CH], F32, tag="pn", bufs=1)
                        for nb in range(CH // 512):
                            nc.tensor.matmul(nn[:, nb * 512:(nb + 1) * 512],
                                             ones[:],
                                             sq[:, nb * 512:(nb + 1) * 512],
                                             start=True, stop=True)
                        rt = wp.tile([128, CH], F32, tag="rt", bufs=2)
                        nc.scalar.activation(rt[:], nn[:], AF.Sqrt,
                                             bias=0.0, scale=float(HD))
                        inv = wp.tile([128, CH], F32, tag="inv", bufs=2)
                        nc.vector.reciprocal_approx_fast(out=inv[:], in_=rt[:])
                        nc.vector.tensor_mul(nrm[:, c * CH:(c + 1) * CH],
                                             ps[:], inv[:])
                    return nrm

                # ---- phase K: kv path for all heads ----
                for h in range(H):
                    kvn = proj_norm(h, WkvT, "kv")
                    kvc = wp.tile([128, TOK], BF16, tag="kvc", bufs=2)
                    kvci = wp.tile([128, TOK], BF16, tag="kvci", bufs=2)
                    for dst, dft in ((kvc, cm), (kvci, sn)):
                        for c in range(TOK // CH):
                            pf = pp.tile([128, CH], F32, tag="pf", bufs=1)
                            for nb in range(CH // 512):
                                col = c * CH + nb * 512
                                nc.tensor.matmul(
                                    pf[:, nb * 512:(nb + 1) * 512], dft[:],
                                    kvn[:, col:col + 512],
                                    start=True, stop=True)
                            init = 0.0 if c == 0 else dst[:, c * CH - 1:c * CH]
                            nc.vector.tensor_tensor_scan(
                                dst[:, c * CH:(c + 1) * CH], pf[:], zeros[:],
                                init, ALU.add, ALU.add)
                    nc.vector.tensor_copy(loc[:, 2 * h:2 * h + 1],
                                          kvc[:, TOK - 1:TOK])
                    nc.vector.tensor_copy(loc[:, 2 * h + 1:2 * h + 2],
                                          kvci[:, TOK - 1:TOK])
                    nc.sync.dma_start(kvc_d[h], kvc[:])
                    nc.sync.dma_start(kvci_d[h], kvci[:])

                # ---- cross-core cumsum prefix (pair AllGather) ----
                nc.sync.dma_start(cc_in[:], loc[:])
                nc.gpsimd.collective_compute(
                    "AllGather", ALU.bypass,
                    replica_groups=[[0, 1], [2, 3], [4, 5], [6, 7]],
                    ins=[cc_in.opt()], outs=[cc_out.opt()])
                nc.sync.dma_start(part[:], cc_out[0:HD, :])
                nc.vector.tensor_mul(offs[:], part[:], mask[:])

                # ---- phase Q+M: q path, complex mult, ifft per head ----
                for h in range(H):
                    qn = proj_norm(h, WqT, "q")
                    qfr = wp.tile([128, TOK], BF16, tag="qfr", bufs=2)
                    qfi = wp.tile([128, TOK], BF16, tag="qfi", bufs=2)
                    for dst, dft in ((qfr, cm), (qfi, sn)):
                        for c in range(TOK // CH):
                            pf = pp.tile([128, CH], F32, tag="pf", bufs=1)
                            for nb in range(CH // 512):
                                col = c * CH + nb * 512
                                nc.tensor.matmul(
                                    pf[:, nb * 512:(nb + 1) * 512], dft[:],
                                    qn[:, col:col + 512],
                                    start=True, stop=True)
                            nc.scalar.copy(dst[:, c * CH:(c + 1) * CH], pf[:])
                    kr = wp.tile([128, TOK], BF16, tag="kr", bufs=2)
                    ki = wp.tile([128, TOK], BF16, tag="ki", bufs=2)
                    nc.sync.dma_start(kr[:], kvc_d[h])
                    nc.sync.dma_start(ki[:], kvci_d[h])
                    o_re = offs[:, 2 * h:2 * h + 1]
                    o_im = offs[:, 2 * h + 1:2 * h + 2]
                    qvr = wp.tile([128, TOK], BF16, tag="qvr")
                    qvi = wp.tile([128, TOK], BF16, tag="qvi")
                    t1 = wp.tile([128, TOK], BF16, tag="tmp", bufs=2)
                    t2 = wp.tile([128, TOK], BF16, tag="tmp", bufs=2)
                    nc.vector.scalar_tensor_tensor(t1[:], kr[:], o_re, qfr[:],
                                                   ALU.add, ALU.mult)
                    nc.vector.scalar_tensor_tensor(t2[:], ki[:], o_im, qfi[:],
                                                   ALU.add, ALU.mult)
                    nc.vector.tensor_sub(qvr[:], t1[:], t2[:])
                    t3 = wp.tile([128, TOK], BF16, tag="tmp", bufs=2)
                    t4 = wp.tile([128, TOK], BF16, tag="tmp", bufs=2)
                    nc.vector.scalar_tensor_tensor(t3[:], ki[:], o_im, qfr[:],
                                                   ALU.add, ALU.mult)
                    nc.vector.scalar_tensor_tensor(t4[:], kr[:], o_re, qfi[:],
                                                   ALU.add, ALU.mult)
                    nc.vector.tensor_add(qvi[:], t3[:], t4[:])
                    vh = wp.tile([128, TOK], BF16, tag="vh", bufs=2)
                    for c in range(TOK // CH):
                        pv = pp.tile([128, CH], F32, tag="pf", bufs=1)
                        for nb in range(CH // 512):
                            col = c * CH + nb * 512
                            nc.tensor.matmul(pv[:, nb * 512:(nb + 1) * 512],
                                             am[:], qvr[:, col:col + 512],
                                             start=True, stop=False)
                            nc.tensor.matmul(pv[:, nb * 512:(nb + 1) * 512],
                                             bn[:], qvi[:, col:col + 512],
                                             start=False, stop=True)
                        nc.scalar.copy(vh[:, c * CH:(c + 1) * CH], pv[:])
                    nc.sync.dma_start(vh_d[h], vh[:])

            # ---- phase O: output projection ----
            with (
                tc.tile_pool(name="po", bufs=1) as op,
                tc.tile_pool(name="pso", bufs=1, space="PSUM") as pp2,
            ):
                wo = op.tile([128, H * D], BF16)
                for h in range(H):
                    nc.sync.dma_start(wo[:, h * D:(h + 1) * D],
                                      WoT[h * 128:(h + 1) * 128, :])
                for j in range(TOK // 128):
                    vsl = op.tile([128, H * 128], BF16, tag="vsl", bufs=3)
                    nc.sync.dma_start(
                        vsl[:].rearrange("p (h s) -> p h s", h=H),
                        vh_d[:, :, j * 128:(j + 1) * 128].rearrange(
                            "h p s -> p h s"))
                    pos = [pp2.tile([128, 512], F32, tag="po", bufs=8,
                                    name=f"po_{j}_{d_}")
                           for d_ in range(4)]
                    for h in range(H):
                        lt = vsl[:, h * 128:(h + 1) * 128]
                        for d_ in range(4):
                            nc.tensor.matmul(
                                pos[d_][:], lt,
                                wo[:, h * D + d_ * 512: h * D + d_ * 512 + 512],
                                start=(h == 0), stop=(h == H - 1))
                    osb = op.tile([128, D], F32, tag="osb", bufs=2)
                    for d_ in range(4):
                        nc.scalar.copy(osb[:, d_ * 512:(d_ + 1) * 512],
                                       pos[d_][:])
                    nc.sync.dma_start(out[j * 128:(j + 1) * 128, :], osb[:])
    nc.compile()
    return nc


def _consts():
    n = np.arange(HD)
    ang = 2 * np.pi * np.outer(n, n) / HD
    Cm = np.cos(ang).astype(np.float32)
    Sm = np.sin(ang).astype(np.float32)
    return {
        "Cm": Cm.astype(BF), "Sn": (-Sm).astype(BF),
        "Am": (Cm / HD).astype(BF), "Bn": (-Sm / HD).astype(BF),
        "ones": np.ones((HD, HD), dtype=BF),
    }


def kernel(x, W_q, W_kv, W_o):
    if "nc" not in _CACHE:
        _CACHE["nc"] = _build()
    nc = _CACHE["nc"]
    shared = _consts()
    shared["WqT"] = np.ascontiguousarray(W_q.T).astype(BF)
    shared["WkvT"] = np.ascontiguousarray(W_kv.T).astype(BF)
    shared["WoT"] = np.ascontiguousarray(W_o.T).astype(BF)
    in_maps = []
    for c in range(N_CORES):
        b, hf = c // 2, c % 2
        chunk = x[b, hf * TOK:(hf + 1) * TOK, :]
        m = dict(shared)
        m["xT"] = np.ascontiguousarray(chunk.T).astype(BF)
        m["mask"] = (np.ones if hf else np.zeros)((HD, 2 * H), dtype=np.float32)
        in_maps.append(m)
    res = run_bass_kernel_spmd(nc, in_maps, core_ids=list(range(N_CORES)))
    out = np.empty((B, S, D), dtype=np.float32)
    for c in range(N_CORES):
        b, hf = c // 2, c % 2
        out[b, hf * TOK:(hf + 1) * TOK, :] = res.results[c]["out"]
    return out
